# revision 40
# baseline (speedup 1.0000x reference)
"""CrystalEncoder Trainium2 kernel.

Strategy: pure data parallel — one crystal (batch element) per NeuronCore.
All O(N^2) work (pairwise distances, RBF expansion, gated message passing)
runs on-device in a single fused Bass/Tile kernel; the host only does O(N)
input prep (embedding gather, operand packing) and the final (B,H)->(B,LAT)
projections.

Key structural ideas (vs the obvious implementation):
  * The gate tensor is SYMMETRIC in (i,j): d_ij = d_ji, so
    gate[i,j,h] = gate[j,i,h]. Only the lower block-triangle (j < i0+8 for
    the 8-row block at i0) is ever computed; each computed block feeds the
    aggregation twice:  agg[i] += gate_ij*hm[j]   (reduce over j, "D1") and
    agg[j] += gate_ij*hm[i]  ("D2", strict rectangle j < i0 only).  This
    halves PE gate matmuls and both ACT softplus passes (softplus = Exp
    then Ln; this toolchain has no native Softplus table).
  * rbfT is PREFIX-PACKED: block b (8 i-rows, jlen = 8b+8, groups at
    partition 0/64) lives at a packed offset, so matmul/ACT/DVE access
    patterns stay contiguous and no work is spent on j > i0+8.
  * The RBF table needs d at every bin partition; a DMA broadcast is
    pathologically slow (single-partition-read replication ran at
    ~5 GB/s), so d is broadcast by a K=1 fp32 ones-matmul on the PE
    (exact), then ACT does t2 = Square(sqrt(g)*d - sqrt(g)*c_k) with a
    per-partition bias and rbf = Exp(-t2) -> bf16.  No cancellation:
    the old K=4 expanded-exponent matmul needed full fp32 (4 cyc/col).
  * Stage-2 table production is INTERLEAVED with layer-0 consumption
    block-by-block (LAG=1), sharing one bufs=2 PSUM pool, so PE/ACT/DVE
    pipeline across the stages instead of running them serially.
  * DVE specifics: bf16 multiplies run at 2x; reduces are fixed-rate
    (1 elem/cycle) and strided reduces are 1.6x slower, so D2 uses a
    contiguous pairwise bf16 add-tree over the 8-row dim and D1
    pre-halves in bf16 before the f32-rate reduce.

Device dataflow per core (N=256 atoms, H=128, BINS=40, NL=2):
  1. D2[i,j] via one K=5 fp32 matmul (two 128-row i-tiles), Relu, Sqrt.
  2. Per block: DMA-gather packed d prefix (HWDGE on the idle SP engine),
     PE ones-matmul broadcast, ACT Square + Exp -> packed bf16 rbfT.
  3. Per layer, per 8-row block: gate matmul (K=40 bf16, edge_w
     stationary) -> PSUM bank slots; Exp+Ln (softplus) -> bf16;
     D1 mul + halve + reduce; D2 mul + add-tree + accumulate into accJ;
     node update via two accumulating K=128 f32 matmuls (aggT + accJ) +
     Silu + mask.
  4. Pooling: reduce over atoms -> sum_h [H, 1] -> DRAM.
Host: g = sum_h / (n_valid + 1e-6); mu / log_var projections.

Measured on HW (NTFF profile, core 0): 328,614 ns vs 405,128 ns for the
previous staged kernel (rel err 2.9e-3, gate 2e-2).

Sync discipline: this walrus build supports at most ONE semaphore wait per
instruction; _install_wait_splitter() splits multi-wait instructions with
same-engine nop carriers.
"""

import numpy as np
import ml_dtypes

B, N, H, LAT, NL, BINS = 8, 256, 128, 64, 2, 40
VMAX = 8.0
GAMMA = 1.0 / (VMAX / BINS) ** 2  # 25.0

G = 2                  # 40-bin groups at partition offsets 0 / 64
IPG = N // G           # 128 i-rows per group
NBLK = N // 8          # 8-row blocks, global index b: i0 = 8*b
RPF = 16               # i-rows per stage-2 fill (2 blocks)
NFILL = IPG // RPF     # fills per group-range (8)


def _blk(b):
    """Geometry of block b: (group, local row, j-extent, packed offset)."""
    g = b // (IPG // 8)
    loc = b * 8 - g * IPG          # local i-row within group
    jlen = b * 8 + 8               # D1 j-range [0, jlen)
    return g, loc, jlen


_OFF = {}


def _off(b):
    """Packed free offset of block b inside its group's rbfT range."""
    if b not in _OFF:
        g = b // (IPG // 8)
        off = 0
        for bb in range(g * (IPG // 8), b):
            off += 8 * _blk(bb)[2]
        _OFF[b] = off
    return _OFF[b]


LOCF = max(_off(b) + 8 * _blk(b)[2] for b in range(NBLK))  # packed size

# Block processing order: alternate big (group 1) and small (group 0)
# blocks so per-iteration pipeline work is uniform from the start.
ORDER = []
for _i in range(NBLK // 2):
    ORDER.append(NBLK // 2 + _i)
    ORDER.append(_i)

_CACHE = {}


def _install_wait_splitter():
    """This walrus build supports at most ONE semaphore wait per ISA
    instruction. Split every multi-wait instruction by inserting same-engine
    NoOp carriers, each holding one of the waits, immediately before it.
    Semantics are preserved: the engine executes its stream in order, so all
    original wait conditions still hold before the instruction runs."""
    import bass_rust
    import concourse.tile as tile
    from concourse import mybir

    if getattr(tile.TileContext, "_wait_split_installed", False):
        return
    orig = tile.TileContext._lower_ordered_insts
    counter = [0]

    def patched(self, ordered):
        for insts in ordered.values():
            newl = []
            for inst in insts:
                si = inst.sync_info
                ow = list(si.on_wait) if (si is not None and si.on_wait) else []
                if len(ow) > 1 and inst.engine != mybir.EngineType.Unassigned:
                    for w in ow[:-1]:
                        counter[0] += 1
                        nop = bass_rust.InstNoOp(
                            name=f"wsplit_{counter[0]}", ins=[], outs=[]
                        )
                        nop.engine = inst.engine
                        nop.sync_info = bass_rust.SyncInfo(
                            on_wait=[w], on_update=[]
                        )
                        newl.append(nop)
                    inst.sync_info = bass_rust.SyncInfo(
                        on_wait=[ow[-1]], on_update=list(si.on_update or [])
                    )
                newl.append(inst)
            insts[:] = newl
        return orig(self, ordered)

    tile.TileContext._lower_ordered_insts = patched

    def patched_dab(self, tick_clock, wait_clock):
        # Reimplementation of _drain_and_barrier: the kernel-tail drain
        # otherwise carries one wait per proc. Emit single-wait SP nop
        # carriers covering the global clock, then a bare drain.
        from concourse.vector_clock import ScopedClock

        probe = self.nc.sync.nop()
        wait_clock.add_sem_waits(
            probe.ins, ScopedClock({None: tick_clock.global_clock})
        )
        si = probe.ins.sync_info
        ow = list(si.on_wait) if (si is not None and si.on_wait) else []
        if len(ow) > 1:
            probe.ins.sync_info = bass_rust.SyncInfo(
                on_wait=[ow[0]], on_update=list(si.on_update or [])
            )
            for w in ow[1:]:
                n2 = self.nc.sync.nop()
                n2.ins.sync_info = bass_rust.SyncInfo(on_wait=[w], on_update=[])
        self.nc.sync.drain()
        self.nc.all_engine_barrier()
        popped = self.nc._tile_sem_poison_stack.pop()
        assert popped is self._sem_poison
        self.nc.clear_and_free_semaphores(list(self.sems.allocated().values()))
        self.nc.all_engine_barrier()

    tile.TileContext._drain_and_barrier = patched_dab
    tile.TileContext._wait_split_installed = True


def _build_nc(reps=1):
    import concourse.bass as bass
    import concourse.tile as tile
    from concourse import mybir

    _install_wait_splitter()

    F32 = mybir.dt.float32
    BF16 = mybir.dt.bfloat16
    AF = mybir.ActivationFunctionType
    ALU = mybir.AluOpType
    X = mybir.AxisListType.X
    POOL = mybir.EngineType.Pool

    nc = bass.Bass("TRN2", target_bir_lowering=False, debug=False)

    def dep_nop(engine, aps):
        """Engine-local nop reading `aps`: pulls their producers' ticks into
        the engine's observed clock so later real instructions need at most
        one new semaphore wait."""
        nop = engine.nop(hint="dep").ins
        nop.ins = [engine.lower_ap(ap) for ap in aps]
        return nop

    d_geo = nc.dram_tensor("geo", [5, 2 * N], F32, kind="ExternalInput")
    d_h0T = nc.dram_tensor("h0T", [H, N], F32, kind="ExternalInput")
    d_maskF = nc.dram_tensor("maskF", [H, N], F32, kind="ExternalInput")
    d_cb2 = nc.dram_tensor("cb2", [64 * G, 1], F32, kind="ExternalInput")
    d_ones = nc.dram_tensor("ones", [64 * G, 64], F32, kind="ExternalInput")
    d_ewR = nc.dram_tensor("ewR", [64 * G, NL * H], BF16, kind="ExternalInput")
    d_ebT = nc.dram_tensor("ebT", [H, NL], F32, kind="ExternalInput")
    d_nwT = nc.dram_tensor("nwT", [H, NL * H], F32, kind="ExternalInput")
    d_nbT = nc.dram_tensor("nbT", [H, NL], F32, kind="ExternalInput")
    d_sumh = nc.dram_tensor("sumh", [H, 1], F32, kind="ExternalOutput")

    with tile.TileContext(nc) as tc:
        with tc.tile_pool(name="consts", bufs=1) as consts:
            kw = dict(forced_dma_engine=POOL)
            t_geo = consts.tile_from(d_geo[:], **kw)
            t_hT = consts.tile_from(d_h0T[:], **kw)
            t_maskF = consts.tile_from(d_maskF[:], **kw)
            t_cb2 = consts.tile_from(d_cb2[:], **kw)
            t_ones = consts.tile_from(d_ones[:], **kw)
            t_ewR = consts.tile_from(d_ewR[:], **kw)
            t_ebT = consts.tile_from(d_ebT[:], **kw)
            t_nbT = consts.tile_from(d_nbT[:], **kw)
            t_nwT = consts.tile_from(d_nwT[:], **kw)

            rbfT = consts.tile([64 * G, LOCF], BF16)

            # every engine pre-observes the (single) DMA proc at its max tick
            dep_nop(nc.tensor, [t_geo[:], t_ewR[:], t_nwT[:]])
            dep_nop(nc.scalar, [t_cb2[:], t_ebT[:], t_nbT[:]])
            dep_nop(nc.vector, [t_hT[:], t_maskF[:]])
            dep_nop(nc.sync, [t_geo[:]])

            h00 = consts.tile([H, N], mybir.dt.float32, tag="h00")
            nc.vector.tensor_copy(h00[:], t_hT[:])

            for rep in range(reps):
              if rep > 0:
                # restore initial h (body updates t_hT in place)
                nc.vector.tensor_copy(t_hT[:], h00[:])
              # ---- stage 1: pairwise distances ----
              with tc.tile_pool(name="geo", bufs=1) as geo:
                  dst = []
                  with tc.tile_pool(name="geop", bufs=2, space="PSUM") as geop:
                      for it in range(2):
                          d2p = geop.tile([128, N], F32, tag="ps")
                          nc.tensor.matmul(
                              d2p[:], t_geo[:, it * 128:(it + 1) * 128],
                              t_geo[:, N:2 * N], start=True, stop=True,
                          )
                          c = geo.tile([128, N], F32, tag=f"d2c{it}")
                          nc.scalar.activation(c[:], d2p[:], AF.Relu)
                          s = geo.tile([128, N], F32, tag=f"dist{it}")
                          nc.scalar.activation(s[:], c[:], AF.Sqrt)
                          dst.append(s)

                  # ---- stages 2+3 interleaved ----
                  # Stage-2 (rbf table) block production is interleaved with
                  # layer-0 consumption so PE/ACT/DVE all stay busy: per k,
                  # emit stage2(block k) then layer0(block k - LAG). The
                  # stage-2 broadcast PSUM and the gate PSUM share one
                  # bufs=2 pool (4 banks each).
                  SQG = float(np.sqrt(GAMMA))
                  LAG = 1
                  with tc.tile_pool(name="rfp", bufs=2) as rfp, \
                       tc.tile_pool(name="upool", bufs=2) as upool, \
                       tc.tile_pool(name="lay", bufs=1) as lay, \
                       tc.tile_pool(name="work", bufs=3) as work, \
                       tc.tile_pool(name="gpp", bufs=2, space="PSUM") as gpp:

                      def emit_stage2(gb):
                          # gather packed d prefix -> one partition;
                          # broadcast to 40 bin partitions via K=1 fp32
                          # ones-matmul (exact); Square + Exp on ACT.
                          g, loc, jlen = _blk(gb)
                          p0 = 64 * g
                          o0 = _off(gb)
                          w = 8 * jlen
                          df = rfp.tile([64 * G, 8 * N], F32, tag="df")
                          nc.sync.dma_start(
                              out=df[p0:p0 + 1, :w],
                              in_=dst[g][loc:loc + 8, :jlen],
                          )
                          dep_nop(nc.tensor, [df[:]])
                          bc = gpp.tile([64 * G, 8 * N], F32, tag="gp")
                          for s in range(0, w, 512):
                              sw = min(512, w - s)
                              nc.tensor.matmul(
                                  bc[p0:p0 + BINS, s:s + sw],
                                  t_ones[p0:p0 + 1, :BINS],
                                  df[p0:p0 + 1, s:s + sw],
                                  start=True, stop=True,
                              )
                          u = upool.tile([64 * G, 8 * N], F32, tag="u")
                          nc.scalar.activation(
                              u[p0:p0 + BINS, :w], bc[p0:p0 + BINS, :w],
                              AF.Square,
                              bias=t_cb2[p0:p0 + BINS], scale=SQG,
                          )
                          nc.scalar.activation(
                              rbfT[p0:p0 + BINS, o0:o0 + w],
                              u[p0:p0 + BINS, :w], AF.Exp,
                              bias=0.0, scale=-1.0,
                          )

                      def emit_block(l, b, hmr, aggT, accJ):
                          g, loc, jlen = _blk(b)
                          i0 = 8 * b
                          jr = i0  # D2 strict rectangle j < i0
                          o0 = _off(b)
                          # each 2-row sub-matmul gets its own 512-wide
                          # PSUM bank slot (must not cross bank bounds)
                          gp = gpp.tile([H, 8 * N], F32, tag="gp")
                          for rr in range(0, 8, 2):
                              nc.tensor.matmul(
                                  gp[:, (rr // 2) * 512:
                                     (rr // 2) * 512 + 2 * jlen],
                                  t_ewR[64 * g:64 * g + BINS,
                                        l * H:(l + 1) * H],
                                  rbfT[64 * g:64 * g + BINS,
                                       o0 + rr * jlen:
                                       o0 + (rr + 2) * jlen],
                                  start=True, stop=True,
                              )
                          # softplus(x) = ln(exp(x) + 1); shared table set
                          gx = work.tile([H, 8 * N], BF16, tag="gx")
                          nc.scalar.activation(
                              gx[:, :8 * jlen].rearrange(
                                  "p (s c) -> p s c", c=2 * jlen),
                              gp[:].rearrange(
                                  "p (s c) -> p s c",
                                  c=512)[:, :, :2 * jlen],
                              AF.Exp, bias=t_ebT[:, l:l + 1],
                          )
                          gt = work.tile([H, 8 * N], BF16, tag="gt")
                          nc.scalar.activation(
                              gt[:, :8 * jlen], gx[:, :8 * jlen], AF.Ln,
                              bias=1.0,
                          )
                          gtv = gt[:, :8 * jlen].rearrange(
                              "p (r c) -> p r c", c=jlen)
                          # D1: agg[i] = sum_j gate*hm[j]
                          pp = work.tile([H, 8 * N], BF16, tag="pp")
                          ppv = pp[:, :8 * jlen].rearrange(
                              "p (r c) -> p r c", c=jlen)
                          nc.vector.tensor_mul(
                              ppv, gtv,
                              hmr[:, None, :jlen]
                              .broadcast_to([H, 8, jlen]),
                          )
                          # pre-halve twice in bf16 (2x rate) before the
                          # f32-rate reduce: j-halves are contiguous runs
                          jh = jlen // 2
                          ph = work.tile([H, 4 * N], BF16, tag="ph")
                          phv = ph[:, :8 * jh].rearrange(
                              "p (r c) -> p r c", c=jh)
                          nc.vector.tensor_add(
                              phv, ppv[:, :, :jh], ppv[:, :, jh:],
                          )
                          jq = jh // 2
                          pq = work.tile([H, 2 * N], BF16, tag="pq")
                          pqv = pq[:, :8 * jq].rearrange(
                              "p (r c) -> p r c", c=jq)
                          nc.vector.tensor_add(
                              pqv, phv[:, :, :jq], phv[:, :, jq:],
                          )
                          nc.vector.reduce_sum(
                              out=aggT[:, i0:i0 + 8], in_=pqv, axis=X,
                          )
                          if jr > 0:
                              # D2: agg[j] += sum_{i in block} gate*hm[i]
                              # bf16 mul then contiguous pairwise-add tree
                              # (strided reduce is 1.6x slower than adds)
                              p2 = work.tile([H, 8 * N], BF16, tag="p2")
                              p2v = p2[:, :8 * jr].rearrange(
                                  "p (r c) -> p r c", c=jr)
                              nc.vector.tensor_mul(
                                  p2v,
                                  hmr[:, i0:i0 + 8, None]
                                  .broadcast_to([H, 8, jr]),
                                  gtv[:, :8, :jr],
                              )
                              q1 = work.tile([H, 4 * N], BF16, tag="q1")
                              q1v = q1[:, :4 * jr].rearrange(
                                  "p (r c) -> p r c", c=jr)
                              nc.vector.tensor_add(
                                  q1v, p2v[:, 0:4, :], p2v[:, 4:8, :],
                              )
                              nc.vector.tensor_add(
                                  q1v[:, 0:2, :], q1v[:, 0:2, :],
                                  q1v[:, 2:4, :],
                              )
                              t2 = work.tile([H, N], BF16, tag="t2")
                              nc.vector.tensor_add(
                                  t2[:, :jr], q1v[:, 0, :], q1v[:, 1, :],
                              )
                              nc.vector.tensor_add(
                                  accJ[:, :jr], accJ[:, :jr], t2[:, :jr],
                              )

                      def layer_tail(l, aggT, accJ):
                          dep_nop(nc.tensor, [aggT[:], accJ[:]])
                          zp = gpp.tile([H, 8 * N], F32, tag="gp")
                          nc.tensor.matmul(
                              zp[:, :N], t_nwT[:, l * H:(l + 1) * H],
                              aggT[:], start=True, stop=False,
                          )
                          nc.tensor.matmul(
                              zp[:, :N], t_nwT[:, l * H:(l + 1) * H],
                              accJ[:], start=False, stop=True,
                          )
                          sl = lay.tile([H, N], F32, tag=f"sil{l}")
                          nc.scalar.activation(
                              sl[:], zp[:, :N], AF.Silu,
                              bias=t_nbT[:, l:l + 1],
                          )
                          h2 = lay.tile([H, N], F32, tag=f"h2_{l}")
                          nc.vector.tensor_add(h2[:], t_hT[:], sl[:])
                          nc.vector.tensor_mul(t_hT[:], h2[:], t_maskF[:])

                      hmr = lay.tile([H, N], BF16, tag="hmr0")
                      nc.vector.tensor_copy(hmr[:], t_hT[:])
                      aggT = lay.tile([H, N], F32, tag="agg0")
                      accJ = lay.tile([H, N], F32, tag="accj0")
                      nc.gpsimd.memset(accJ[:], 0.0)
                      # layer 0, interleaved with rbf-table production
                      for k in range(NBLK + LAG):
                          if k < NBLK:
                              emit_stage2(ORDER[k])
                          if k >= LAG:
                              emit_block(0, ORDER[k - LAG], hmr, aggT, accJ)
                      layer_tail(0, aggT, accJ)
                      # layer 1 (rbfT fully resident)
                      hmr = lay.tile([H, N], BF16, tag="hmr1")
                      nc.vector.tensor_copy(hmr[:], t_hT[:])
                      aggT = lay.tile([H, N], F32, tag="agg1")
                      accJ = lay.tile([H, N], F32, tag="accj1")
                      nc.gpsimd.memset(accJ[:], 0.0)
                      for b in ORDER:
                          emit_block(1, b, hmr, aggT, accJ)
                      layer_tail(1, aggT, accJ)

                      sumh = lay.tile([H, 1], F32, tag="sumh")
                      nc.vector.reduce_sum(out=sumh[:], in_=t_hT[:], axis=X)
                      nc.gpsimd.dma_start(out=d_sumh[:], in_=sumh[:])

    return nc


def _get_nc(reps=1):
    key = f"nc{reps}"
    if key not in _CACHE:
        _CACHE[key] = _build_nc(reps)
    return _CACHE[key]


def check_waits(nc, max_waits=1, verbose=True):
    """Report instructions carrying more than `max_waits` semaphore waits."""
    bad = []
    for f in nc.m.functions:
        for bb in f.blocks:
            for ins in bb.instructions:
                si = ins.sync_info
                if si is None:
                    continue
                ow = si.on_wait or []
                if len(ow) > max_waits:
                    bad.append((ins.name, type(ins).__name__, ins.engine,
                                [w.ant_name for w in ow]))
    if verbose:
        for b in bad:
            print("MULTIWAIT:", b)
    return bad


def _shared_inputs(edge_w, edge_b, node_w, node_b):
    centers = np.linspace(0.0, VMAX, BINS).astype(np.float64)
    # groups live at 64-partition-aligned offsets (matmul base-partition rule)
    cb2 = np.zeros((64 * G, 1), np.float32)
    ewR = np.zeros((64 * G, NL * H), np.float32)
    for g in range(G):
        cb2[64 * g:64 * g + BINS, 0] = -np.sqrt(GAMMA) * centers
        for l in range(NL):
            ewR[64 * g:64 * g + BINS, l * H:(l + 1) * H] = edge_w[l]
    ewR = ewR.astype(ml_dtypes.bfloat16)
    ebT = np.ascontiguousarray(edge_b.T).astype(np.float32)      # [H, NL]
    nwT = np.concatenate([node_w[l] for l in range(NL)], axis=1)
    nwT = np.ascontiguousarray(nwT).astype(np.float32)           # [H, NL*H]
    nbT = np.ascontiguousarray(node_b.T).astype(np.float32)      # [H, NL]
    ones = np.ones((64 * G, 64), np.float32)
    return dict(cb2=cb2, ones=ones, ewR=ewR, ebT=ebT, nwT=nwT, nbT=nbT)


def make_in_maps(atom_types, frac_coords, lattice, mask, emb_table,
                 edge_w, edge_b, node_w, node_b):
    shared = _shared_inputs(edge_w, edge_b, node_w, node_b)
    in_maps = []
    for b in range(B):
        cart = (frac_coords[b] @ lattice[b]).astype(np.float32)  # (N, 3)
        nsq = (cart * cart).sum(-1).astype(np.float32)
        # geo[:, :N] = lhsT (-2x, -2y, -2z, 1, |c|^2); geo[:, N:] = rhs
        # (x, y, z, |c|^2 + 1e-6, 1):  D2 = lhsT.T @ rhs
        geo = np.zeros((5, 2 * N), np.float32)
        geo[0, :N] = -2.0 * cart[:, 0]
        geo[1, :N] = -2.0 * cart[:, 1]
        geo[2, :N] = -2.0 * cart[:, 2]
        geo[3, :N] = 1.0
        geo[4, :N] = nsq
        geo[0, N:] = cart[:, 0]
        geo[1, N:] = cart[:, 1]
        geo[2, N:] = cart[:, 2]
        geo[3, N:] = nsq + 1e-6
        geo[4, N:] = 1.0
        types = np.where(mask[b], atom_types[b], 0).astype(np.int64)
        h0T = np.ascontiguousarray(emb_table[types].T).astype(np.float32)
        maskF = np.broadcast_to(
            mask[b].astype(np.float32)[None, :], (H, N)
        ).copy()
        in_maps.append(dict(geo=geo, h0T=h0T, maskF=maskF, **shared))
    return in_maps


def kernel(**inputs):
    from concourse.bass_utils import run_bass_kernel_spmd

    atom_types = np.asarray(inputs["atom_types"])
    frac_coords = np.asarray(inputs["frac_coords"], np.float32)
    lattice = np.asarray(inputs["lattice"], np.float32)
    mask = np.asarray(inputs["mask"]).astype(bool)
    emb_table = np.asarray(inputs["emb_table"], np.float32)
    edge_w = np.asarray(inputs["edge_w"], np.float32)
    edge_b = np.asarray(inputs["edge_b"], np.float32)
    node_w = np.asarray(inputs["node_w"], np.float32)
    node_b = np.asarray(inputs["node_b"], np.float32)
    mu_w = np.asarray(inputs["mu_w"], np.float32)
    mu_b = np.asarray(inputs["mu_b"], np.float32)
    var_w = np.asarray(inputs["var_w"], np.float32)
    var_b = np.asarray(inputs["var_b"], np.float32)

    nc = _get_nc()
    in_maps = make_in_maps(atom_types, frac_coords, lattice, mask, emb_table,
                           edge_w, edge_b, node_w, node_b)
    res = run_bass_kernel_spmd(nc, in_maps, core_ids=list(range(B)))
    sum_h = np.stack([res.results[b]["sumh"][:, 0] for b in range(B)])
    n_valid = mask.sum(1).astype(np.float32)
    g = sum_h / (n_valid[:, None] + 1e-6)
    mu = (g @ mu_w + mu_b).astype(np.float32)
    log_var = (g @ var_w + var_b).astype(np.float32)
    return mu, log_var
